# revision 56
# baseline (speedup 1.0000x reference)
"""Bilinear (softmax-free) multi-head attention on 8 TRN2 NeuronCores.

Math: for each batch b,
    out_b = x_b @ M_b,   M_b = sum_h Wq[h] @ (Wk[h].T @ (x_b.T x_b) @ Wv[h]) @ Wo[h]
since (Q K^T) V = Q (K^T V) and every projection is linear. This collapses the
O(L^2) attention into two L-sized GEMMs (G = x^T x and out = x @ M) plus a tiny
512x512 head-folding chain.

Distribution (SPMD, no collectives): core i handles batch b = i//4 and output
row chunk c = i%4. Each core streams the full x_b to build G redundantly
(cheaper than a cross-core all-reduce at this size), folds all 8 heads into M,
and computes/stores only its own 1024-row slice of out. The per-core x input is
row-rotated so the core's own chunk occupies rows 0..1024 (G is invariant to row
permutations), letting all 8 cores share one program.

Precision/perf notes:
- x ships as fp16 (values ~N(0,1); fp16 matmuls accumulate exactly into fp32
  PSUM) which halves the dominant DMA stream; everything downstream overflows
  fp16 so the chain runs in float32r (fp32 storage, ~11-bit multiply) at full
  PE rate. Measured end-to-end rel err ~2e-4.
- G exploits symmetry: only the upper-triangle 128-row blocks are computed;
  the lower blocks are PE-transposed mirrors (bitwise identical).
- The chain computes D_h^T = (G Wv)_h^T Wk_h via one N=256 window per head
  pair, assembles blockdiag(D_h^T), and folds all heads with dense 512-wide
  matmuls; W layouts are host-prepacked ((d,hk)/(hk,d)/(hk,o)) and the
  own-chunk x^T is host-transposed.
"""

import numpy as np

import concourse.tile as tile
from concourse import bacc, mybir
from concourse.bass_utils import run_bass_kernel_spmd
from concourse.masks import make_identity

F32 = mybir.dt.float32
F32R = mybir.dt.float32r
F16 = mybir.dt.float16

B, L, D = 2, 4096, 512
H, DK = 8, 64
CHUNK = 1024          # output rows per core
P = 128               # SBUF partitions
TL = L // P           # 32 x-tiles of 128 rows
NXD = 8               # x DMA chunks (4 x-tiles per chunk)
N_CORES = 8

_CACHE = {}


def _build():
    nc = bacc.Bacc("TRN2", target_bir_lowering=False, debug=False)

    x_d = nc.dram_tensor("x", [L, D], F16, kind="ExternalInput").ap()
    xt_d = nc.dram_tensor("xt", [D, CHUNK], F32R, kind="ExternalInput").ap()
    wk_d = nc.dram_tensor("wk", [D, D], F16, kind="ExternalInput").ap()    # (d, h*k)
    wv_d = nc.dram_tensor("wv", [D, D], F16, kind="ExternalInput").ap()    # (d, h*k)
    wqt_d = nc.dram_tensor("wqt", [D, D], F32R, kind="ExternalInput").ap()  # (h*k, d)
    wo_d = nc.dram_tensor("wo", [D, D], F32R, kind="ExternalInput").ap()    # (h*k, o)
    out_d = nc.dram_tensor("out", [CHUNK, D], F32, kind="ExternalOutput").ap()

    with tile.TileContext(nc) as tc:
        import contextlib

        with contextlib.ExitStack() as ctx:
            consts = ctx.enter_context(tc.tile_pool(name="consts", bufs=1))
            wpool = ctx.enter_context(tc.tile_pool(name="wpool", bufs=1))
            xpool = ctx.enter_context(tc.tile_pool(name="xpool", bufs=1))
            spool = ctx.enter_context(tc.tile_pool(name="spool", bufs=1))
            opool = ctx.enter_context(tc.tile_pool(name="opool", bufs=6))
            pg = ctx.enter_context(tc.tile_pool(name="pg", bufs=4, space="PSUM"))
            pt = ctx.enter_context(tc.tile_pool(name="pt", bufs=4, space="PSUM"))

            ident_f32 = consts.tile([P, P], F32, tag="identf")
            make_identity(nc, ident_f32)
            ident = consts.tile([P, P], F16, tag="ident")
            nc.vector.tensor_copy(ident[:], ident_f32[:])

            # Zero-filled block-diag D^T holder, prepared off the critical path;
            # the FP phase later writes only the 8 diagonal 64x64 blocks.
            zero_f32 = consts.tile([P, 2048], F32, tag="zerof")
            nc.gpsimd.memset(zero_f32[:], 0.0)
            dtbd = spool.tile([P, 2048], F32R, tag="dt", name="dtbd")
            nc.vector.tensor_copy(dtbd[:], zero_f32[:])

            # --- x first: 8 SBUF tiles of [128, 2048]; tile j holds rows 512j..
            # x_sb[j][p, 512*tt + d] = x[128*(4j+tt) + p, d] ---
            xr = x_d.rearrange("(t p) d -> p t d", p=P)  # [128, 32, 512]
            x_sb = []
            for j in range(NXD):
                xt_ = xpool.tile([P, 2048], F16, tag=f"x{j}", name=f"x_sb{j}")
                if j == 0:  # split early chunks so the PE ramps sooner
                    for hh in range(4):
                        nc.sync.dma_start(
                            out=xt_.rearrange("p (t d) -> p t d", t=4)[:, hh:hh + 1, :],
                            in_=xr[:, hh:hh + 1, :],
                        )
                else:
                    for hh in range(2):
                        nc.sync.dma_start(
                            out=xt_.rearrange("p (t d) -> p t d", t=4)[:, 2 * hh:2 * hh + 2, :],
                            in_=xr[:, j * 4 + 2 * hh:j * 4 + 2 * hh + 2, :],
                        )
                x_sb.append(xt_)

            # --- weights (needed only from the B phase on):
            # W_sb[p, cb*512 + j] = W[128*cb + p, j] ---
            wk_sb = wpool.tile([P, 2048], F16, tag="wk", name="wk_sb")
            wv_sb = wpool.tile([P, 2048], F16, tag="wv", name="wv_sb")
            wqt_sb = wpool.tile([P, 2048], F32R, tag="wqt", name="wqt_sb")
            wo_sb = wpool.tile([P, 2048], F32R, tag="wo", name="wo_sb")
            for sb, dram in ((wv_sb, wv_d), (wk_sb, wk_d), (wo_sb, wo_d), (wqt_sb, wqt_d)):
                nc.sync.dma_start(
                    out=sb.rearrange("p (c j) -> p c j", c=4),
                    in_=dram.rearrange("(c p) j -> p c j", p=P),
                )

            def xtile(t):
                return x_sb[t // 4][:, (t % 4) * 512:(t % 4 + 1) * 512]


            # --- own-chunk x^T, host-prepared: xt_sb[p, 1024*kc + l] = x[l, 128kc+p]
            xt_sb = spool.tile([P, 4096], F32R, tag="xt", name="xt_sb")
            nc.sync.dma_start(
                out=xt_sb.rearrange("p (kc l) -> p kc l", kc=4),
                in_=xt_d.rearrange("(kc p) l -> p kc l", p=P),
            )

            # --- G = x^T x (512x512, symmetric): row-block m computes only
            # columns >= 128m (fp16 matmuls have no min-N penalty); missing
            # lower blocks are mirrored via PE transpose afterwards. ---
            g_n0 = [0, 128, 256, 384]  # first computed column per m-block (fp16: any N)
            g_ps = []
            for m in range(4):
                g_ps.append(pg.tile([P, 512], F32, tag="acc", name=f"g_ps{m}"))
            for t in range(TL):
                xt_ = xtile(t)
                for m in range(4):
                    n0 = g_n0[m]
                    nc.tensor.matmul(
                        g_ps[m][:, n0:512],
                        lhsT=xt_[:, m * P:(m + 1) * P],
                        rhs=xt_[:, n0:512],
                        start=(t == 0),
                        stop=(t == TL - 1),
                    )
            g_sb = spool.tile([P, 2048], F16, tag="g", name="g_sb")

            def g_copy(m):
                n0 = g_n0[m]
                nc.vector.tensor_copy(
                    g_sb[:, m * 512 + n0:(m + 1) * 512], g_ps[m][:, n0:512]
                )

            def g_mirror(mr, jc):
                # G[mr-block, jc-cols] = T(G[jc-block, mr-cols])
                mir_ps = pt.tile([P, 512], F16, tag="tp", name="mir_ps")
                nc.tensor.transpose(
                    mir_ps[:, 0:P],
                    g_sb[:, jc * 512 + mr * P:jc * 512 + (mr + 1) * P],
                    ident[:],
                )
                nc.vector.tensor_copy(
                    g_sb[:, mr * 512 + jc * P:mr * 512 + (jc + 1) * P],
                    mir_ps[:, 0:P],
                )

            g_copy(0)
            g_mirror(1, 0)
            g_mirror(2, 0)
            g_mirror(3, 0)
            g_copy(1)
            g_mirror(2, 1)
            g_mirror(3, 1)
            g_copy(2)
            g_mirror(3, 2)
            g_copy(3)

            # --- B = G @ Wv_all (512 x 512). lhsT uses G symmetry. ---
            b_ps = []
            for m in range(4):
                b_ps.append(pg.tile([P, 512], F32, tag="acc", name=f"b_ps{m}"))
            for m in range(4):
                for kc in range(4):
                    nc.tensor.matmul(
                        b_ps[m][:],
                        lhsT=g_sb[:, kc * 512 + m * P: kc * 512 + (m + 1) * P],
                        rhs=wv_sb[:, kc * 512:(kc + 1) * 512],
                        start=(kc == 0),
                        stop=(kc == 3),
                    )
            b_sb = spool.tile([P, 2048], F16, tag="b", name="b_sb")
            for m in range(4):
                nc.vector.tensor_copy(b_sb[:, m * 512:(m + 1) * 512], b_ps[m][:])

            # --- FP = B^T @ Wk_all (512x512); diag 64-blocks are D_h^T,
            # copied into the pre-zeroed block-diag tile. ---
            ns_sb = spool.tile([P, 2048], F32R, tag="ns", name="ns_sb")

            def fp_phase(m):  # heads 2m (partitions 0:64), 2m+1 (64:128)
                # fp16 inputs run full rate at any N: only the 128 diag cols
                fp_ps = pt.tile([P, 128], F32, tag="tp", name="fp_ps")
                for kc in range(4):
                    nc.tensor.matmul(
                        fp_ps[:],
                        lhsT=b_sb[:, kc * 512 + m * P: kc * 512 + (m + 1) * P],
                        rhs=wk_sb[:, kc * 512 + 128 * m: kc * 512 + 128 * m + 128],
                        start=(kc == 0),
                        stop=(kc == 3),
                    )
                h0, h1 = 2 * m, 2 * m + 1
                nc.vector.tensor_copy(
                    dtbd[0:64, m * 512 + 64 * h0: m * 512 + 64 * h0 + 64],
                    fp_ps[0:64, 0:64],
                )
                nc.vector.tensor_copy(
                    dtbd[64:128, m * 512 + 64 * h1: m * 512 + 64 * h1 + 64],
                    fp_ps[64:128, 64:128],
                )

            def n_phase(m):
                # N_stack = blockdiag(D_h) @ Wo_stack: diagonal chunk only
                n_ps = pt.tile([P, 512], F32, tag="tp", name="n_ps")
                nc.tensor.matmul(
                    n_ps[:],
                    lhsT=dtbd[:, m * 512 + P * m: m * 512 + P * (m + 1)],
                    rhs=wo_sb[:, m * 512:(m + 1) * 512],
                    start=True,
                    stop=True,
                )
                nc.vector.tensor_copy(ns_sb[:, m * 512:(m + 1) * 512], n_ps[:])

            # --- M = WqT_stack^T-contract @ N_stack:  M[d, o] ---
            m_sb = spool.tile([P, 2048], F32R, tag="m", name="m_sb")
            m_ps = []
            for m in range(4):
                m_ps.append(pg.tile([P, 512], F32, tag="acc", name=f"m_ps{m}"))

            # staggered so DVE copies land while the PE runs the next group
            fp_phase(0)
            fp_phase(1)
            n_phase(0)
            fp_phase(2)
            n_phase(1)
            fp_phase(3)
            n_phase(2)
            n_phase(3)
            for m in range(4):
                for kc in range(4):
                    nc.tensor.matmul(
                        m_ps[m][:],
                        lhsT=wqt_sb[:, kc * 512 + m * P: kc * 512 + (m + 1) * P],
                        rhs=ns_sb[:, kc * 512:(kc + 1) * 512],
                        start=(kc == 0),
                        stop=(kc == 3),
                    )
            for m in range(4):
                nc.vector.tensor_copy(m_sb[:, m * 512:(m + 1) * 512], m_ps[m][:])

            # --- out chunk = x[0:1024] @ M ---
            for lb in range(CHUNK // P):
                o_ps = pg.tile([P, 512], F32, tag="acc", name="o_ps")
                for kc in range(4):
                    nc.tensor.matmul(
                        o_ps[:],
                        lhsT=xt_sb[:, 1024 * kc + P * lb: 1024 * kc + P * (lb + 1)],
                        rhs=m_sb[:, kc * 512:(kc + 1) * 512],
                        start=(kc == 0),
                        stop=(kc == 3),
                    )
                o_sb = opool.tile([P, 512], F32, tag="o", name="o_sb")
                if lb == CHUNK // P - 1:
                    for hx in range(2):
                        nc.vector.tensor_copy(
                            o_sb[:, hx * 256:(hx + 1) * 256], o_ps[:, hx * 256:(hx + 1) * 256]
                        )
                        nc.sync.dma_start(
                            out=out_d[lb * P:(lb + 1) * P, hx * 256:(hx + 1) * 256],
                            in_=o_sb[:, hx * 256:(hx + 1) * 256],
                        )
                else:
                    nc.vector.tensor_copy(o_sb[:], o_ps[:])
                    nc.sync.dma_start(out=out_d[lb * P:(lb + 1) * P, :], in_=o_sb[:])

    nc.compile()
    return nc


def _get_nc():
    if "nc" not in _CACHE:
        _CACHE["nc"] = _build()
    return _CACHE["nc"]


def kernel(x, W_q, W_k, W_v, W_o):
    x = np.ascontiguousarray(np.asarray(x, np.float32))
    W_q = np.asarray(W_q, np.float32)
    W_k = np.asarray(W_k, np.float32)
    W_v = np.asarray(W_v, np.float32)
    W_o = np.asarray(W_o, np.float32)

    wk_all = np.ascontiguousarray(W_k.transpose(1, 0, 2).reshape(D, D).astype(np.float16))
    wv_all = np.ascontiguousarray(W_v.transpose(1, 0, 2).reshape(D, D).astype(np.float16))
    wqt = np.ascontiguousarray(W_q.transpose(0, 2, 1).reshape(D, D))      # (hk, d)
    wo = np.ascontiguousarray(W_o.reshape(D, D))                          # (hk, o)

    nc = _get_nc()
    in_maps = []
    for i in range(N_CORES):
        b, c = divmod(i, 4)
        xb = np.roll(x[b], -c * CHUNK, axis=0).astype(np.float16)  # G is perm-invariant
        xt = np.ascontiguousarray(x[b, c * CHUNK:(c + 1) * CHUNK].T)  # (D, CHUNK)
        in_maps.append(
            {"x": np.ascontiguousarray(xb), "xt": xt, "wk": wk_all, "wv": wv_all,
             "wqt": wqt, "wo": wo}
        )

    res = run_bass_kernel_spmd(nc, in_maps, list(range(N_CORES)))

    out = np.empty((B, L, D), np.float32)
    for i in range(N_CORES):
        b, c = divmod(i, 4)
        out[b, c * CHUNK:(c + 1) * CHUNK] = res.results[i]["out"]
    return out
